# revision 48
# baseline (speedup 1.0000x reference)
"""Trainium2 Bass kernel for LoRA multi-head attention.

Computation (per batch b):
    q = x @ Wq + bw0 * (x @ la_q) @ lb_q        (same for k, v)
    attn = softmax((q_h @ k_h^T) / sqrt(64))    per head h (12 heads, hd=64)
    out  = attn @ v_h                           -> concat heads
    y    = out @ Wp + bp
Sharding: batch-parallel — 8 batches, one per NeuronCore. Weights replicated.

Design (driven by the TimelineSim cost model; ~158us vs 197.5us baseline):
  - LoRA is folded on the host: x@W + bw*(x@la)@lb = x@(W + bw*la@lb), a
    rank-16 weight update — the whole on-chip LoRA path disappears.
  - The attention softmax is ACT-bound: 96 exp instructions over the full
    12 x 1024^2 score set are a hard ~96us floor on the Activation
    engine. Everything else hides inside that window: projection chunk
    groups are emitted in sub-2us pieces spread across the two heads
    preceding the pair that needs them (the exp pipeline only buffers ~2
    chunks of PSUM), v-production rides in head 1's chunk loop, and the
    output projection runs in three passes (dc 0-2 during heads 6-8,
    dc 3 during heads 10-11, dc 4-5 pipelined against the final
    normalize tail).
  - Projections are bf16 512-col matmul groups (engine-bound; many small
    fp8-DoubleRow matmuls are SEQ-decode-bound at ~123ns each and lose).
    Weights are pre-scaled x16 on the host so PSUM holds 16*q — that
    keeps the fp8 hi/lo pair split of q/k clear of e4m3's subnormal
    floor. The x16 cancels exactly: exp's scale absorbs 1/256 of q*k and
    the softmax ratio cancels v's 16 (its fused ones column is 16).
  - QK^T is one fp8e4 DoubleRow matmul per (jc, nh) at 0.5 cycles/row:
    k is stored [k_hi; k_lo] stacked across the 128 partitions (the lhsT
    plane dim is a 0-stride AP reading the stack twice), q as duplicated
    (q_hi, q_lo) planes, so a single instruction computes the fully
    compensated (k_hi+k_lo)^T (q_hi+q_lo) product — 2x over f32r at
    ~2^-8 relative precision. The split runs on DVE (hi copy + sub from
    the 16x PSUM); layout assembly is SBUF->SBUF partition-moving DMAs.
  - exp stays bf16: plain-fp8 exp measurably fails the 2e-2 gate at this
    score variance (sigma~2 concentrates softmax weights; 3.6% e4m3
    noise on the top weights is fatal). PV and the output projection are
    bf16. Softmax is unnormalized with a fused ones-column denominator;
    normalize tails (DVE reciprocal + Pool partition_broadcast + bf16
    multiply, Pool crossing-copy for odd head halves) are deferred into
    the next head's instruction stream. GPSIMD ops never touch PSUM
    (BIR verifier constraint).
"""

import os
from contextlib import ExitStack

import numpy as np

import concourse.bacc as bacc
import concourse.bass as bass
import concourse.mybir as mybir
import concourse.tile as tile
from concourse.bass_utils import run_bass_kernel_spmd

F32 = mybir.dt.float32
F32R = mybir.dt.float32r
BF16 = mybir.dt.bfloat16
F8 = mybir.dt.float8e4
DR = mybir.MatmulPerfMode.DoubleRow

C = 768          # model dim
NI = 1024        # sequence length
H = 12           # heads
HD = 64          # head dim
R = 16           # LoRA rank
KC = C // 128    # 6 contraction chunks
IC = NI // 128   # 8 sequence chunks
SCALE = HD ** -0.5
SW = 16.0        # host pre-scale on Wq/Wk/Wv and la
N_CORES = 8

_CACHE = {}


def build_nc():
    nc = bacc.Bacc("TRN2", target_bir_lowering=False, debug=False)

    xbf_d = nc.dram_tensor("xbf", [C, NI], BF16, kind="ExternalInput").ap()
    w_d = {
        nm: nc.dram_tensor(f"w_{nm}", [128, KC, C], BF16, kind="ExternalInput").ap()
        for nm in ("q", "k", "v", "p")
    }
    bp_d = nc.dram_tensor("bp", [128, KC], F32, kind="ExternalInput").ap()
    yT_d = nc.dram_tensor("yT", [C, NI], BF16, kind="ExternalOutput").ap()

    def mm(out, *, lhsT, rhs, start, stop, perf_mode=None):
        return nc.tensor.matmul(
            out, lhsT=lhsT, rhs=rhs, start=start, stop=stop, perf_mode=perf_mode
        )

    with tile.TileContext(nc) as tc, ExitStack() as ctx:
        ctx.enter_context(
            nc.allow_low_precision(reason="fp8 pair-compensated QK + bf16 attention")
        )
        persist = ctx.enter_context(tc.tile_pool(name="persist", bufs=1))

        xbf_sb = persist.tile([128, KC, NI], BF16, name="xbf_sb")
        w_sb = {
            nm: persist.tile([128, KC, C], BF16, name=f"w{nm}_sb")
            for nm in ("q", "k", "v", "p")
        }
        bp_sb = persist.tile([128, KC], F32, name="bp_sb")

        # dependency-ordered input DMAs, chunk-granular so the first
        # projection groups pipeline against the loads
        xr = xbf_d.rearrange("(kc p) n -> p kc n", p=128)
        for g in range(3):
            kcs = slice(2 * g, 2 * g + 2)
            nc.sync.dma_start(out=xbf_sb[:, kcs, :], in_=xr[:, kcs, :])
            nc.sync.dma_start(out=w_sb["q"][:, kcs, :], in_=w_d["q"][:, kcs, :])
            nc.sync.dma_start(out=w_sb["k"][:, kcs, :], in_=w_d["k"][:, kcs, :])

        def late_input_dmas():
            for nm in ("v", "p"):
                for hhalf in range(2):
                    kcs = slice(hhalf * 3, hhalf * 3 + 3)
                    nc.sync.dma_start(
                        out=w_sb[nm][:, kcs, :], in_=w_d[nm][:, kcs, :]
                    )
            nc.sync.dma_start(out=bp_sb, in_=bp_d)

        q_all = persist.tile([128, H, 2, NI], F8, name="q_all")
        k_all = persist.tile([128, H, NI], F8, name="k_all")
        vS = persist.tile([128, IC, H * (HD + 1)], BF16, name="vS")
        oT = [
            persist.tile([128, NI], BF16, name=f"oT{dc}", tag=f"oT{dc}")
            for dc in range(KC)
        ]
        ones_f32 = persist.tile([128, IC * H], F32, name="ones_f32")
        nc.vector.memset(ones_f32, SW)
        warm = persist.tile([1, 8], BF16, name="warm")
        nc.scalar.activation(
            out=warm,
            in_=ones_f32[0:1, 0:8],
            func=mybir.ActivationFunctionType.Exp,
            scale=0.01,
        )
        v_ones = vS.rearrange("p i (h x) -> p i h x", x=HD + 1)[:, :, :, HD : HD + 1]
        nc.vector.tensor_copy(
            out=v_ones,
            in_=ones_f32.rearrange("p (i h o) -> p i h o", i=IC, h=H, o=1),
        )

        psA = ctx.enter_context(tc.tile_pool(name="psA", bufs=2, space="PSUM"))
        psB = ctx.enter_context(tc.tile_pool(name="psB", bufs=2, space="PSUM"))
        psO = ctx.enter_context(tc.tile_pool(name="psO", bufs=1, space="PSUM"))
        stg = ctx.enter_context(tc.tile_pool(name="stg", bufs=2))
        spool = ctx.enter_context(tc.tile_pool(name="spool", bufs=6))
        rpool = ctx.enter_context(tc.tile_pool(name="rpool", bufs=1))
        bpool = ctx.enter_context(tc.tile_pool(name="bpool", bufs=1))
        tpool = ctx.enter_context(tc.tile_pool(name="tpool", bufs=1))
        ospool = ctx.enter_context(tc.tile_pool(name="ospool", bufs=3))
        ypool = ctx.enter_context(tc.tile_pool(name="ypool", bufs=1))
        dpool = ctx.enter_context(tc.tile_pool(name="dstage", bufs=2, space="DRAM"))

        stg_tiles = {}

        def proj_piece(pj, nm, dc, nh):
            """One [128,512] projection group of q/k chunk dc + fp8 split."""
            if nh == 0:
                stg_tiles[(nm, dc)] = (
                    stg.tile([128, NI], F8, name=f"{nm}hi{dc}", tag=f"hi{pj}"),
                    stg.tile([128, NI], F8, name=f"{nm}lo{dc}", tag=f"lo{pj}"),
                )
            hi_st, lo_st = stg_tiles[(nm, dc)]
            cols = slice(nh * 512, (nh + 1) * 512)
            ps = psB.tile([128, 512], F32, name="pj", tag="ps")
            for kc in range(KC):
                mm(
                    ps,
                    lhsT=w_sb[nm][:, kc, dc * 128 : (dc + 1) * 128],
                    rhs=xbf_sb[:, kc, cols],
                    start=(kc == 0),
                    stop=(kc == KC - 1),
                )
            nc.vector.tensor_copy(out=hi_st[:, cols], in_=ps)
            nc.vector.tensor_sub(out=lo_st[:, cols], in0=ps, in1=hi_st[:, cols])

        def assemble_q(dc, p):
            qhi, qlo = stg_tiles[("q", dc)]
            h = 2 * dc + p
            rows = slice(p * 64, p * 64 + 64)
            for half in range(2):
                dst = slice(half * 64, half * 64 + 64)
                nc.sync.dma_start(out=q_all[dst, h, 0, :], in_=qhi[rows, :])
                nc.sync.dma_start(out=q_all[dst, h, 1, :], in_=qlo[rows, :])

        def assemble_k(dc, p):
            khi, klo = stg_tiles[("k", dc)]
            h = 2 * dc + p
            rows = slice(p * 64, p * 64 + 64)
            nc.sync.dma_start(out=k_all[0:64, h, :], in_=khi[rows, :])
            nc.sync.dma_start(out=k_all[64:128, h, :], in_=klo[rows, :])

        def assemble_head(dc, p):
            """SBUF->SBUF DMAs building head 2dc+p's QK tiles."""
            assemble_q(dc, p)
            assemble_k(dc, p)

        def assemble(dc):
            assemble_head(dc, 1)
            assemble_head(dc, 0)

        def v_group(ic):
            """v chunk ic (16x, bf16) into the augmented vS layout."""
            for (lo, hi), (h0, h1) in (((0, 512), (0, 8)), ((512, 768), (8, 12))):
                ps = psB.tile([128, hi - lo], F32, name="v_ps", tag="ps")
                for kc in range(KC):
                    mm(
                        ps,
                        lhsT=xbf_sb[:, kc, ic * 128 : (ic + 1) * 128],
                        rhs=w_sb["v"][:, kc, lo:hi],
                        start=(kc == 0),
                        stop=(kc == KC - 1),
                    )
                nc.vector.tensor_copy(
                    out=vS[:, ic, :].rearrange("p (h x) -> p h x", x=HD + 1)[
                        :, h0:h1, 0:HD
                    ],
                    in_=ps.rearrange("p (h d) -> p h d", d=HD),
                )

        def emit_tail(h, o_sb):
            """Normalize head h: reciprocal of the 16z row, DRAM-staged
            partition broadcast, bf16 multiply into oT (Pool crossing-copy
            for the odd half)."""
            dc, half = divmod(h, 2)
            half *= HD
            r_sb = rpool.tile([1, NI], BF16, name="r_sb", tag="r_sb")
            b_sb = bpool.tile([HD, NI], BF16, name="b_sb", tag="b_sb")
            nc.vector.reciprocal(out=r_sb, in_=o_sb[HD : HD + 1, :])
            nc.gpsimd.partition_broadcast(b_sb, r_sb, channels=HD)
            if half == 0:
                nc.vector.tensor_mul(
                    out=oT[dc][0:HD, :], in0=o_sb[0:HD, :], in1=b_sb
                )
            else:
                tmp = tpool.tile([HD, NI], BF16, name="tmp", tag="tmp")
                nc.vector.tensor_mul(out=tmp, in0=o_sb[0:HD, :], in1=b_sb)
                nc.gpsimd.tensor_copy(out=oT[dc][HD:128, :], in_=tmp)

        y_sbs = [None] * KC
        ybf_sbs = [None] * KC

        def emit_y(ec, dcs, mode):
            """One output-projection pass for chunk ec over oT[dcs].
            mode: 'first' (bias add into f32 accumulator), 'mid' (f32 add),
            'last' (add + round once to the bf16 output tile)."""
            for nh in range(2):
                cols = slice(nh * 512, (nh + 1) * 512)
                y_ps = psB.tile([128, 512], F32, name="y_ps", tag="ps")
                for kc in dcs:
                    mm(
                        y_ps,
                        lhsT=w_sb["p"][:, kc, ec * 128 : (ec + 1) * 128],
                        rhs=oT[kc][:, cols],
                        start=(kc == dcs[0]),
                        stop=(kc == dcs[-1]),
                    )
                if mode == "first":
                    if y_sbs[ec] is None:
                        y_sbs[ec] = ypool.tile(
                            [128, NI], F32, name=f"y_sb{ec}", tag=f"y_sb{ec}"
                        )
                    nc.vector.tensor_scalar_add(
                        out=y_sbs[ec][:, cols], in0=y_ps,
                        scalar1=bp_sb[:, ec : ec + 1],
                    )
                elif mode == "mid":
                    nc.vector.tensor_add(
                        out=y_sbs[ec][:, cols], in0=y_sbs[ec][:, cols], in1=y_ps
                    )
                else:
                    if ybf_sbs[ec] is None:
                        ybf_sbs[ec] = ypool.tile(
                            [128, NI], BF16, name=f"ybf{ec}", tag=f"ybf{ec}"
                        )
                    nc.vector.tensor_add(
                        out=ybf_sbs[ec][:, cols], in0=y_sbs[ec][:, cols], in1=y_ps
                    )

        # ----------------- interleaved schedule ------------------------
        late_input_dmas()

        # dc0 runs while the x/w loads drip in: interleave the q and k
        # matmuls per chunk so k's work fills q's DMA-wait gaps
        def proj_pair_dc0(nh):
            cols = slice(nh * 512, (nh + 1) * 512)
            if nh == 0:
                for nm, pj in (("q", 0), ("k", 1)):
                    stg_tiles[(nm, 0)] = (
                        stg.tile([128, NI], F8, name=f"{nm}hi0", tag=f"hi{pj}"),
                        stg.tile([128, NI], F8, name=f"{nm}lo0", tag=f"lo{pj}"),
                    )
            psq = psB.tile([128, 512], F32, name="pjq", tag="ps")
            psk = psB.tile([128, 512], F32, name="pjk", tag="ps")
            for kc in range(KC):
                for nm, ps in (("q", psq), ("k", psk)):
                    mm(
                        ps,
                        lhsT=w_sb[nm][:, kc, 0:128],
                        rhs=xbf_sb[:, kc, cols],
                        start=(kc == 0),
                        stop=(kc == KC - 1),
                    )
            for nm, ps in (("q", psq), ("k", psk)):
                hi_st, lo_st = stg_tiles[(nm, 0)]
                nc.vector.tensor_copy(out=hi_st[:, cols], in_=ps)
                nc.vector.tensor_sub(
                    out=lo_st[:, cols], in0=ps, in1=hi_st[:, cols]
                )

        proj_pair_dc0(0)
        proj_pair_dc0(1)
        assemble_q(0, 1)
        assemble_k(0, 1)

        # per (head index, jc) -> list of thunks to emit mid-stream
        inserts = {}

        def add_insert(i, jc, fn):
            inserts.setdefault((i, jc), []).append(fn)

        # qk projection pieces for dc=1..5: dc1 squeezed into head 1 (it is
        # needed by head 2); dc>=2 get four pieces on even head 2d-2 and
        # their assembly DMAs (PE-free) on odd head 2d-1
        add_insert(1, 1, lambda: proj_piece(0, "q", 1, 0))
        add_insert(1, 2, lambda: proj_piece(0, "q", 1, 1))
        add_insert(1, 3, lambda: proj_piece(1, "k", 1, 0))
        add_insert(1, 4, lambda: proj_piece(1, "k", 1, 1))
        add_insert(0, 1, lambda: assemble_head(0, 0))
        add_insert(1, 5, lambda: assemble(1))
        for d in range(2, KC):
            i0 = 2 * d - 2
            add_insert(i0, 1, lambda d=d: proj_piece(0, "q", d, 0))
            add_insert(i0, 3, lambda d=d: proj_piece(0, "q", d, 1))
            add_insert(i0, 5, lambda d=d: proj_piece(1, "k", d, 0))
            add_insert(i0, 7, lambda d=d: proj_piece(1, "k", d, 1))
            add_insert(i0 + 1, 1, lambda d=d: assemble(d))
        # y passes on odd/late heads (collision-free with qk pieces, each
        # emitted only after its oT chunks are certainly normalized):
        # main3 (dc 0..2) on heads 7/9, mid3 (dc3) on heads 10/11,
        # dc4+dc5 after the final tail
        for n, (iy, jy) in enumerate(
            [(7, 3), (7, 5), (7, 7), (9, 3), (9, 5), (9, 7)]
        ):
            add_insert(iy, jy, lambda n=n: emit_y(n, [0, 1, 2], "first"))
        for n, (iy, jy) in enumerate(
            [(10, 2), (10, 4), (10, 6), (11, 2), (11, 4), (11, 6)]
        ):
            add_insert(iy, jy, lambda n=n: emit_y(n, [3], "mid"))

        head_order = [2 * dc + p for dc in range(KC) for p in (1, 0)]
        fin = {}

        def final_chain(h, o_ps, s):
            """Normalize chain for column half s of the final head (DVE/Pool
            only — emitted between the two PV halves without stalling PE)."""
            if s == 0:
                fin["o"] = ospool.tile([HD + 1, NI], BF16, name="fo_sb", tag="o_sb")
                fin["r"] = rpool.tile([1, NI], BF16, name="r_sbf", tag="r_sb")
                fin["b"] = bpool.tile([HD, NI], BF16, name="b_sbf", tag="b_sb")
            fo_sb, fr_sb, fb_sb = fin["o"], fin["r"], fin["b"]
            dc = h // 2
            cols = slice(s * 512, (s + 1) * 512)
            nc.vector.tensor_copy(out=fo_sb[:, cols], in_=o_ps)
            nc.vector.reciprocal(
                out=fr_sb[:, cols], in_=fo_sb[HD : HD + 1, cols]
            )
            nc.gpsimd.partition_broadcast(
                fb_sb[:, cols], fr_sb[:, cols], channels=HD
            )
            nc.vector.tensor_mul(
                out=oT[dc][0:HD, cols], in0=fo_sb[0:HD, cols], in1=fb_sb[:, cols]
            )

        def final_fix_both():
            """dc4+dc5 output-projection fixup, per-ec over both column
            halves so each chunk's store DMA fires as early as possible.
            PSUM slots alternate between psB and the attention-idle psA pool
            so the matmuls never wait on the adds draining."""
            for ec in range(KC):
                if ybf_sbs[ec] is None:
                    ybf_sbs[ec] = ypool.tile(
                        [128, NI], BF16, name=f"ybf{ec}", tag=f"ybf{ec}"
                    )
                for s in range(2):
                    cols = slice(s * 512, (s + 1) * 512)
                    pool = psA if (2 * ec + s) % 2 else psB
                    y_ps = pool.tile(
                        [128, 512], F32, name="f_ps",
                        tag="s_ps" if (2 * ec + s) % 2 else "ps",
                    )
                    for kc in (4, 5):
                        mm(
                            y_ps,
                            lhsT=w_sb["p"][:, kc, ec * 128 : (ec + 1) * 128],
                            rhs=oT[kc][:, cols],
                            start=(kc == 4),
                            stop=(kc == 5),
                        )
                    nc.vector.tensor_add(
                        out=ybf_sbs[ec][:, cols], in0=y_sbs[ec][:, cols], in1=y_ps
                    )
                nc.sync.dma_start(
                    out=yT_d[ec * 128 : (ec + 1) * 128, :], in_=ybf_sbs[ec]
                )

        pending = None
        for i, h in enumerate(head_order):
            o_ps = [
                psO.tile([HD + 1, 512], F32, name=f"o_ps{nh}", tag=f"o_ps{nh}")
                for nh in range(2)
            ]
            o_sb = ospool.tile([HD + 1, NI], BF16, name="o_sb", tag="o_sb")
            for jc in range(IC):
                s_ps = psA.tile([128, NI], F32, name="s_ps", tag="s_ps")
                k_lhsT = bass.AP(
                    tensor=k_all.tensor,
                    offset=k_all.offset + h * NI + jc * 128,
                    ap=[[k_all.ap[0][0], 128], [0, 2], [1, 128]],
                )
                for nh in range(2):
                    mm(
                        s_ps[:, nh * 512 : (nh + 1) * 512],
                        lhsT=k_lhsT,
                        rhs=q_all[:, h, :, nh * 512 : (nh + 1) * 512],
                        start=True,
                        stop=True,
                        perf_mode=DR,
                    )
                s_sb = spool.tile([128, NI], BF16, name="s_sb", tag="s_sb")
                nc.scalar.activation(
                    out=s_sb,
                    in_=s_ps,
                    func=mybir.ActivationFunctionType.Exp,
                    scale=SCALE / 256.0,
                )
                if i == 0:
                    v_group(jc)
                last_h = i == len(head_order) - 1
                for nh in range(2):
                    mm(
                        o_ps[nh],
                        lhsT=vS[:, jc, h * (HD + 1) : (h + 1) * (HD + 1)],
                        rhs=s_sb[:, nh * 512 : (nh + 1) * 512],
                        start=(jc == 0),
                        stop=(jc == IC - 1),
                    )
                    if jc == IC - 1:
                        # evict this half now so the slot frees for the
                        # next head while the other half still accumulates
                        if last_h:
                            final_chain(h, o_ps[nh], nh)
                        else:
                            nc.vector.tensor_copy(
                                out=o_sb[:, nh * 512 : (nh + 1) * 512],
                                in_=o_ps[nh],
                            )
                if jc == 0 and pending is not None:
                    emit_tail(*pending)
                    pending = None
                for fn in inserts.get((i, jc), ()):
                    fn()
            if i < len(head_order) - 1:
                pending = (h, o_sb)
        final_fix_both()
    nc.compile()
    return nc


def get_nc():
    if "nc" not in _CACHE:
        _CACHE["nc"] = build_nc()
    return _CACHE["nc"]


def _f32r(a):
    import ml_dtypes

    a = np.asarray(a, np.float32)
    hi = a.astype(ml_dtypes.bfloat16).astype(np.float32)
    lo = (a - hi).astype(ml_dtypes.bfloat16).astype(np.float32)
    return hi + lo


def _bf_image(W, scale, cols):
    import ml_dtypes

    return np.ascontiguousarray(
        (scale * np.asarray(W, np.float32))
        .reshape(KC, 128, cols)
        .transpose(1, 0, 2)
    ).astype(ml_dtypes.bfloat16)


def make_in_maps(inputs):
    import ml_dtypes

    x = np.asarray(inputs["x"], np.float32)
    bw = np.asarray(inputs["block_weight"], np.float32)

    def fold(W, la, lb, w):
        # LoRA is a rank-R weight update: x@W + w*(x@la)@lb = x@(W + w*la@lb)
        return np.asarray(W, np.float32) + w * (
            np.asarray(la, np.float32) @ np.asarray(lb, np.float32)
        )

    common = {
        "w_q": _bf_image(fold(inputs["Wq"], inputs["la_q"], inputs["lb_q"], bw[0]), SW, C),
        "w_k": _bf_image(fold(inputs["Wk"], inputs["la_k"], inputs["lb_k"], bw[1]), SW, C),
        "w_v": _bf_image(fold(inputs["Wv"], inputs["la_v"], inputs["lb_v"], bw[2]), SW, C),
        "w_p": _bf_image(inputs["Wp"], 1.0, C),
        "bp": np.ascontiguousarray(
            np.asarray(inputs["bp"], np.float32).reshape(KC, 128).T
        ),
    }
    in_maps = []
    for b in range(N_CORES):
        m = dict(common)
        m["xbf"] = np.ascontiguousarray(x[b].T).astype(ml_dtypes.bfloat16)
        in_maps.append(m)
    return in_maps


def kernel(**inputs):
    nc = get_nc()
    in_maps = make_in_maps(inputs)
    trace = os.environ.get("KBENCH_TRACE", "0") not in ("", "0")
    res = run_bass_kernel_spmd(
        nc, in_maps, core_ids=list(range(N_CORES)), trace=trace
    )
    _CACHE["last_results"] = res
    y = np.stack(
        [np.asarray(res.results[b]["yT"], np.float32).T for b in range(N_CORES)],
        axis=0,
    )
    return np.ascontiguousarray(y.astype(np.float32))
